# revision 1
# baseline (speedup 1.0000x reference)
"""AttnJGNN Trainium2 kernel: 8-core SPMD, pull-model sharded tables.

Structure per iteration (x-table chunk-interleaved across 8 cores):
  TC-A  owners serve gat1 node rows (dma_gather from slice) -> AllToAll
  TC-B  per-cluster attention (transposed-score formulation) -> h rows,
        re-gather into dest-owner buckets -> AllToAll
  TC-C  dest-sorted one-hot seg-sum matmuls + gat1 epilogue (invcnt, Wo, bo,
        residual) writes slice; cf partial sums by cluster -> AllReduce
  TC-D  QKV on cf, per-edge attention, expansion to shared vars -> AllToAll
  TC-E  one-hot seg-sum into x_var slice chunks
Readout: masked column-sum matmuls -> AllReduce -> tanh/MLP on every core.
"""
import numpy as np

N = 50000
M = 100000
C = 1000
VC = 64
KC = 128
E = 8000
S = 8
D = 64
HID = 64
NEG = 0.2
GAMMA = 1.0
ITERS = 3
ACTIVE = 4

NCORE = 8
ROWS = N + M                      # 150000
NCHG = (ROWS + 127) // 128        # 1172 global chunks
TPAD = NCHG * 128                 # 150016
NCH = (NCHG + NCORE - 1) // NCORE  # 147 chunks per core (padded)
SL = NCH * 128                    # 18816 slice rows
CPC = C // NCORE                  # 125 clusters per core
EPC = E // NCORE                  # 1000 edges per core
NNOD = VC + KC                    # 192 nodes per cluster
HB = CPC * 256                    # 32000 h_buf rows
GCAP = 12032                      # dma_gather per-call index cap


def _r128(x):
    return (int(x) + 127) // 128 * 128


def _r16(x):
    return (int(x) + 15) // 16 * 16


def _pack16(idx, width=None):
    """Pack flat int index list -> [128, ceil(n/16)] int16 (16-wrapped,
    replicated across the 8 16-partition groups)."""
    idx = np.asarray(idx)
    n = len(idx)
    cols = (n + 15) // 16
    if width is None:
        width = cols
    buf = np.zeros(cols * 16, np.int16)
    buf[:n] = idx.astype(np.int16)
    arr = np.zeros((128, width), np.int16)
    w = buf.reshape(cols, 16).T
    for g in range(8):
        arr[g * 16:(g + 1) * 16, :cols] = w
    return arr


def _owner(r):
    return (r >> 7) % NCORE


def _local(r):
    return ((r >> 7) // NCORE) * 128 + (r & 127)


def _plan(inp):
    """All host-side planning. Returns (dims, per_core list of dicts)."""
    xv = np.asarray(inp['x_var'], np.float32)
    xc = np.asarray(inp['x_clause'], np.float32)
    sat = np.asarray(inp['satisfaction_scores'], np.float32)
    cvar = np.asarray(inp['cluster_var_ids']).astype(np.int64)
    ccla = np.asarray(inp['cluster_clause_ids']).astype(np.int64)
    ei = np.asarray(inp['cluster_edge_index']).astype(np.int64)
    sv = np.asarray(inp['shared_vars']).astype(np.int64)

    xall = np.zeros((TPAD, D), np.float32)
    xall[:N] = xv
    xall[N:ROWS] = xc

    nodes = np.concatenate([cvar, ccla + N], axis=1)      # [C, 192]

    dims = {}
    per = [dict() for _ in range(NCORE)]

    # ---- x slices ----
    for k in range(NCORE):
        sl = np.zeros((SL, D), np.float32)
        for p in range(NCH):
            ch = p * NCORE + k
            if ch < NCHG:
                sl[p * 128:(p + 1) * 128] = xall[ch * 128:(ch + 1) * 128]
        per[k]['xsl0'] = sl

    # ---- gat1 serve (owners -> consumers) ----
    # L1[o][k]: local rows in consumer-k scan order
    L1 = [[None] * NCORE for _ in range(NCORE)]
    for k in range(NCORE):
        flat = nodes[k * CPC:(k + 1) * CPC].reshape(-1)   # [24000] scan order
        o = _owner(flat)
        lo = _local(flat)
        for j in range(NCORE):
            L1[j][k] = lo[o == j]
    B1 = _r128(max(len(L1[j][k]) for j in range(NCORE) for k in range(NCORE)))
    dims['B1'] = B1
    for j in range(NCORE):
        idx = np.zeros(NCORE * B1, np.int64)
        for k in range(NCORE):
            idx[k * B1:k * B1 + len(L1[j][k])] = L1[j][k]
        per[j]['serve1'] = _pack16(idx)

    # ---- gat1 hop2 (consumer gathers cluster tiles from recv1) ----
    # position g = cl*256 + r (r<192 real); recv1 row = o*B1 + rank
    for k in range(NCORE):
        flat = nodes[k * CPC:(k + 1) * CPC].reshape(-1)
        o = _owner(flat)
        g2 = np.zeros(CPC * 256, np.int64)
        cnt = np.zeros(NCORE, np.int64)
        ranks = np.zeros(len(flat), np.int64)
        for j in range(NCORE):
            m = o == j
            ranks[m] = j * B1 + np.arange(m.sum())
        g2r = ranks.reshape(CPC, NNOD)
        g2 = np.zeros((CPC, 256), np.int64)
        g2[:, :NNOD] = g2r
        per[k]['hop2'] = _pack16(g2.reshape(-1))

    # ---- gat1 contributions: h_buf -> dest-owner buckets ----
    L2len = np.zeros((NCORE, NCORE), np.int64)
    L2 = [[None] * NCORE for _ in range(NCORE)]   # [k][o2] -> (hrows, dloc)
    for k in range(NCORE):
        flat = nodes[k * CPC:(k + 1) * CPC].reshape(-1)
        o2 = _owner(flat)
        dloc = _local(flat)
        g = (np.arange(CPC)[:, None] * 256 + np.arange(NNOD)[None, :]).reshape(-1)
        for j in range(NCORE):
            m = o2 == j
            L2[k][j] = (g[m], dloc[m])
            L2len[k, j] = m.sum()
    B2 = _r128(L2len.max())
    dims['B2'] = B2
    for k in range(NCORE):
        idx = np.zeros(NCORE * B2, np.int64)
        for j in range(NCORE):
            idx[j * B2:j * B2 + len(L2[k][j][0])] = L2[k][j][0]
        per[k]['send2g'] = _pack16(idx)

    # ---- gat1 reduce plan (owner side) ----
    # entries: (dest_local, recv2row = src*B2 + i), sorted by dest
    cnt1 = np.zeros((NCORE, NCH), np.int64)
    ent = []
    for j in range(NCORE):
        ds, rs = [], []
        for s_ in range(NCORE):
            g, dl = L2[s_][j]
            ds.append(dl)
            rs.append(s_ * B2 + np.arange(len(g)))
        ds = np.concatenate(ds)
        rs = np.concatenate(rs)
        srt = np.argsort(ds, kind='stable')
        ds, rs = ds[srt], rs[srt]
        ent.append((ds, rs))
        np.add.at(cnt1[j], ds >> 7, 1)
    BUD1 = np.maximum(_r16(0) + 16, (np.ceil(cnt1.max(0) / 16) * 16).astype(np.int64))
    tot1 = int(BUD1.sum())
    tot1p = _r128(tot1)
    dims['NR2'] = tot1p
    lo1 = np.zeros(NCH + 1, np.int64)
    lo1[1:] = np.cumsum(BUD1)
    dims['LO1'] = lo1
    for j in range(NCORE):
        ds, rs = ent[j]
        gidx = np.zeros(tot1p, np.int64)
        dmod = np.full(tot1p, -1.0, np.float32)
        chs = (ds >> 7).astype(np.int64)
        for ch in range(NCH):
            m = chs == ch
            nn_ = int(m.sum())
            gidx[lo1[ch]:lo1[ch] + nn_] = rs[m]
            dmod[lo1[ch]:lo1[ch] + nn_] = ds[m].astype(np.float32)
        per[j]['r2g'] = _pack16(gidx)
        per[j]['dmod1'] = dmod.reshape(-1, 128).T.copy()   # [128, tot1p/128]
        # invcnt
        cntfull = np.zeros(SL, np.float32)
        np.add.at(cntfull, ent[j][0], 1.0)
        inv = 1.0 / np.maximum(cntfull, 1.0)
        per[j]['invcnt'] = inv.reshape(NCH, 128).T.copy()  # [128, NCH]

    # ---- cf plan: member instances by owner, seg-sum by cluster ----
    NCC = (C + 127) // 128                                 # 8 cluster chunks
    mrow = cvar.reshape(-1)                                # [64000] scan (c,m)
    mc = np.repeat(np.arange(C), VC)
    mo = _owner(mrow)
    ml = _local(mrow)
    cntc = np.zeros((NCORE, NCC), np.int64)
    cfent = []
    for j in range(NCORE):
        m = mo == j
        cs, ls = mc[m], ml[m]                              # sorted by c already
        cfent.append((cs, ls))
        np.add.at(cntc[j], cs >> 7, 1)
    BUDC = (np.ceil(cntc.max(0) / 16) * 16).astype(np.int64) + 16
    totc = int(BUDC.sum())
    totcp = _r128(totc)
    dims['NCF'] = totcp
    loc_ = np.zeros(NCC + 1, np.int64)
    loc_[1:] = np.cumsum(BUDC)
    dims['LOC'] = loc_
    dims['NCC'] = NCC
    for j in range(NCORE):
        cs, ls = cfent[j]
        gidx = np.zeros(totcp, np.int64)
        dmod = np.full(totcp, -1.0, np.float32)
        chs = cs >> 7
        for ch in range(NCC):
            m = chs == ch
            nn_ = int(m.sum())
            gidx[loc_[ch]:loc_[ch] + nn_] = ls[m]
            dmod[loc_[ch]:loc_[ch] + nn_] = cs[m].astype(np.float32)
        per[j]['cfg'] = _pack16(gidx)
        per[j]['dmodc'] = dmod.reshape(-1, 128).T.copy()

    # ---- gat2 edges ----
    for k in range(NCORE):
        c1 = ei[0, k * EPC:(k + 1) * EPC]
        c2 = ei[1, k * EPC:(k + 1) * EPC]
        per[k]['qeg'] = _pack16(np.concatenate([c1, np.zeros(24, np.int64)]))
        per[k]['keg'] = _pack16(np.concatenate([c2, np.zeros(24, np.int64)]))

    # expansion: pairs (e_local, s) -> dest var row
    L3len = np.zeros((NCORE, NCORE), np.int64)
    L3 = [[None] * NCORE for _ in range(NCORE)]
    for k in range(NCORE):
        dsts = sv[k * EPC:(k + 1) * EPC].reshape(-1)       # [8000]
        el = np.repeat(np.arange(EPC), S)
        o3 = _owner(dsts)
        dl = _local(dsts)
        for j in range(NCORE):
            m = o3 == j
            L3[k][j] = (el[m], dl[m])
            L3len[k, j] = m.sum()
    B3 = _r128(L3len.max())
    dims['B3'] = B3
    for k in range(NCORE):
        idx = np.zeros(NCORE * B3, np.int64)
        for j in range(NCORE):
            idx[j * B3:j * B3 + len(L3[k][j][0])] = L3[k][j][0]
        per[k]['send3g'] = _pack16(idx)

    # gat2 reduce plan: var chunks only
    NVCH = 0
    for k in range(NCORE):
        nv = sum(1 for p in range(NCH) if (p * NCORE + k) * 128 < N)
        NVCH = max(NVCH, nv)
    dims['NVCH'] = NVCH
    cnt3 = np.zeros((NCORE, NVCH), np.int64)
    ent3 = []
    for j in range(NCORE):
        ds, rs = [], []
        for s_ in range(NCORE):
            el, dl = L3[s_][j]
            ds.append(dl)
            rs.append(s_ * B3 + np.arange(len(el)))
        ds = np.concatenate(ds)
        rs = np.concatenate(rs)
        srt = np.argsort(ds, kind='stable')
        ds, rs = ds[srt], rs[srt]
        ent3.append((ds, rs))
        np.add.at(cnt3[j], ds >> 7, 1)
    BUD3 = (np.ceil(cnt3.max(0) / 16) * 16).astype(np.int64) + 16
    tot3 = int(BUD3.sum())
    tot3p = _r128(tot3)
    dims['NR3'] = tot3p
    lo3 = np.zeros(NVCH + 1, np.int64)
    lo3[1:] = np.cumsum(BUD3)
    dims['LO3'] = lo3
    for j in range(NCORE):
        ds, rs = ent3[j]
        gidx = np.zeros(tot3p, np.int64)
        dmod = np.full(tot3p, -1.0, np.float32)
        chs = ds >> 7
        for ch in range(NVCH):
            m = chs == ch
            nn_ = int(m.sum())
            gidx[lo3[ch]:lo3[ch] + nn_] = rs[m]
            dmod[lo3[ch]:lo3[ch] + nn_] = ds[m].astype(np.float32)
        per[j]['r3g'] = _pack16(gidx)
        per[j]['dmod3'] = dmod.reshape(-1, 128).T.copy()

    # ---- per-cluster bias columns (key-partition layout) ----
    for k in range(NCORE):
        b = np.zeros((CPC, NNOD), np.float32)
        b[:, VC:] = GAMMA * sat[ccla[k * CPC:(k + 1) * CPC]]
        per[k]['biasA'] = b[:, :128].T.copy()      # [128, CPC]
        per[k]['biasB'] = b[:, 128:].T.copy()      # [64, CPC]

    # ---- weights / consts (shared) ----
    hw1m = float(np.asarray(inp['hw1'])[:ACTIVE].mean())
    hw2m = float(np.asarray(inp['hw2'])[:ACTIVE].mean())
    WQ1 = np.asarray(inp['WQ1'], np.float32)
    WK1 = np.asarray(inp['WK1'], np.float32)
    WV1 = np.asarray(inp['WV1'], np.float32)
    Wo = np.asarray(inp['Wo'], np.float32)
    bo = np.asarray(inp['bo'], np.float32)
    WQE = (WQ1.T / np.sqrt(D)).astype(np.float32)
    WKE = WK1.T.astype(np.float32).copy()
    WVE = (WV1.T * hw1m).astype(np.float32)
    WoTe = Wo.T.astype(np.float32).copy()
    bo_bc = np.tile(bo[None, :], (128, 1)).astype(np.float32)
    shared = {
        'WQE': WQE, 'WKE': WKE, 'WVE': WVE, 'WoTe': WoTe, 'bo_bc': bo_bc,
        'Wq2T': (np.asarray(inp['WQ2'], np.float32).T / np.sqrt(D)).copy(),
        'Wk2T': np.asarray(inp['WK2'], np.float32).T.copy(),
        'Wv2T': (np.asarray(inp['WV2'], np.float32).T * hw2m).copy(),
        'W1T': np.asarray(inp['W1'], np.float32).T.copy(),      # [128, 64]
        'b1': np.asarray(inp['b1'], np.float32).reshape(HID, 1).copy(),
        'W2T': np.asarray(inp['W2'], np.float32).T.copy(),      # [64, 1]
        'b2': np.asarray(inp['b2'], np.float32).reshape(1, 1).copy(),
        'ident': np.eye(128, dtype=np.float32),
        'iota': np.tile(np.arange(128, dtype=np.float32)[None, :], (128, 1)),
        'scale2': np.array([[1.0 / N], [1.0 / M]], np.float32),
    }

    # ---- readout masks [128, 2*NCH] ----
    for k in range(NCORE):
        mask = np.zeros((128, 2 * NCH), np.float32)
        for p in range(NCH):
            ch = p * NCORE + k
            if ch >= NCHG:
                continue
            rows = np.arange(ch * 128, ch * 128 + 128)
            mask[:, 2 * p] = (rows < N).astype(np.float32)
            mask[:, 2 * p + 1] = ((rows >= N) & (rows < ROWS)).astype(np.float32)
        per[k]['rmask'] = mask

    for k in range(NCORE):
        for key in shared:
            per[k][key] = shared[key]
    return dims, per


def _build(dims):
    import concourse.bass as bass
    import concourse.bacc as bacc
    import concourse.tile as tile
    from concourse import mybir
    from concourse.vector_clock import ScopedClock
    from concourse._compat import cdiv

    f32 = mybir.dt.float32
    i16 = mybir.dt.int16
    AF = mybir.ActivationFunctionType
    ALU = mybir.AluOpType

    B1, B2, B3 = dims['B1'], dims['B2'], dims['B3']
    NR2, NR3, NCF = dims['NR2'], dims['NR3'], dims['NCF']
    LO1, LO3, LOC = dims['LO1'], dims['LO3'], dims['LOC']
    NVCH, NCC = dims['NVCH'], dims['NCC']

    class PhaseTC(tile.TileContext):
        # tail without the DMA-ring-resetting drain (it breaks later
        # collectives); split multi-waits (walrus allows 1 per inst).
        def _drain_and_barrier(self, tick_clock, wait_clock):
            probe = self.nc.sync.nop(nofuse=True)
            wait_clock.add_sem_waits(
                probe.ins, ScopedClock({None: tick_clock.global_clock}))
            waits = list(probe.ins.sync_info.on_wait or [])
            if len(waits) > 1:
                probe.ins.sync_info.on_wait = waits[:1]
                for w in waits[1:]:
                    extra = self.nc.sync.nop(nofuse=True)
                    si = extra.ins.sync_info
                    if si is None:
                        si = mybir.SyncInfo(on_wait=[], on_update=[])
                    si.on_wait = [w]
                    extra.ins.sync_info = si
            self.nc.all_engine_barrier()
            assert self.sems is not None
            popped = self.nc._tile_sem_poison_stack.pop()
            assert popped is self._sem_poison
            self.nc.clear_and_free_semaphores(
                list(self.sems.allocated().values()))
            self.nc.all_engine_barrier()

    nc = bacc.Bacc("TRN2", num_swdge_queues=4)

    def din(name, shape, dtype=f32):
        return nc.dram_tensor(name, list(shape), dtype, kind="ExternalInput")

    xsl0 = din("xsl0", [SL, D])
    serve1 = din("serve1", [128, cdiv(NCORE * B1, 16)], i16)
    hop2 = din("hop2", [128, cdiv(CPC * 256, 16)], i16)
    send2g = din("send2g", [128, cdiv(NCORE * B2, 16)], i16)
    r2g = din("r2g", [128, cdiv(NR2, 16)], i16)
    dmod1 = din("dmod1", [128, NR2 // 128])
    invcnt = din("invcnt", [128, NCH])
    cfg = din("cfg", [128, cdiv(NCF, 16)], i16)
    dmodc = din("dmodc", [128, NCF // 128])
    qeg = din("qeg", [128, 64], i16)
    keg = din("keg", [128, 64], i16)
    send3g = din("send3g", [128, cdiv(NCORE * B3, 16)], i16)
    r3g = din("r3g", [128, cdiv(NR3, 16)], i16)
    dmod3 = din("dmod3", [128, NR3 // 128])
    biasA_d = din("biasA", [128, CPC])
    biasB_d = din("biasB", [64, CPC])
    WQE = din("WQE", [64, 64])
    WKE = din("WKE", [64, 64])
    WVE = din("WVE", [64, 64])
    WoTe = din("WoTe", [64, 64])
    bobc_d = din("bo_bc", [128, 64])
    Wq2T = din("Wq2T", [64, 64])
    Wk2T = din("Wk2T", [64, 64])
    Wv2T = din("Wv2T", [64, 64])
    W1T = din("W1T", [128, HID])
    b1t = din("b1", [HID, 1])
    W2T = din("W2T", [HID, 1])
    b2t = din("b2", [1, 1])
    ident_d = din("ident", [128, 128])
    iota_d = din("iota", [128, 128])
    scale2_d = din("scale2", [2, 1])
    rmask_d = din("rmask", [128, 2 * NCH])

    y = nc.dram_tensor("y", [1, 1], f32, kind="ExternalOutput")

    xw = nc.dram_tensor("xw", [SL, D], f32)
    send1 = nc.dram_tensor("send1", [NCORE * B1, D], f32)
    recv1 = nc.dram_tensor("recv1", [NCORE * B1, D], f32)
    h_buf = nc.dram_tensor("h_buf", [HB, D], f32)
    send2 = nc.dram_tensor("send2", [NCORE * B2, D], f32)
    recv2 = nc.dram_tensor("recv2", [NCORE * B2, D], f32)
    cfpart = nc.dram_tensor("cfpart", [1024, D], f32)
    cfall = nc.dram_tensor("cfall", [1024, D], f32)
    q2b = nc.dram_tensor("q2b", [1024, D], f32)
    k2b = nc.dram_tensor("k2b", [1024, D], f32)
    v2b = nc.dram_tensor("v2b", [1024, D], f32)
    u_buf = nc.dram_tensor("u_buf", [1024, D], f32)
    send3 = nc.dram_tensor("send3", [NCORE * B3, D], f32)
    recv3 = nc.dram_tensor("recv3", [NCORE * B3, D], f32)
    sums_d = nc.dram_tensor("sums_d", [2, D], f32)
    sums_a = nc.dram_tensor("sums_a", [2, D], f32)

    # persistent SBUF constants (outside Tile pools)
    sb = {}
    def sbuf_const(name, shape, dtype=f32):
        sb[name] = nc.alloc_sbuf_tensor(f"c_{name}", list(shape), dtype)
        return sb[name]
    idx_s1 = sbuf_const("s1", [128, cdiv(NCORE * B1, 16)], i16)
    idx_h2 = sbuf_const("h2", [128, cdiv(CPC * 256, 16)], i16)
    idx_s2 = sbuf_const("s2", [128, cdiv(NCORE * B2, 16)], i16)
    idx_r2 = sbuf_const("r2", [128, cdiv(NR2, 16)], i16)
    dmod1_t = sbuf_const("dm1", [128, NR2 // 128])
    inv_t = sbuf_const("inv", [128, NCH])
    idx_cf = sbuf_const("cf", [128, cdiv(NCF, 16)], i16)
    dmodc_t = sbuf_const("dmc", [128, NCF // 128])
    idx_qe = sbuf_const("qe", [128, 64], i16)
    idx_ke = sbuf_const("ke", [128, 64], i16)
    idx_s3 = sbuf_const("s3", [128, cdiv(NCORE * B3, 16)], i16)
    idx_r3 = sbuf_const("r3", [128, cdiv(NR3, 16)], i16)
    dmod3_t = sbuf_const("dm3", [128, NR3 // 128])
    wqe_t = sbuf_const("wqe", [64, 64])
    wke_t = sbuf_const("wke", [64, 64])
    wve_t = sbuf_const("wve", [64, 64])
    wote_t = sbuf_const("wote", [64, 64])
    bobc_t = sbuf_const("bobc", [128, 64])
    biasA_t = sbuf_const("biasA", [128, CPC])
    biasB_t = sbuf_const("biasB", [64, CPC])
    wq2_t = sbuf_const("wq2", [64, 64])
    wk2_t = sbuf_const("wk2", [64, 64])
    wv2_t = sbuf_const("wv2", [64, 64])
    ident_t = sbuf_const("id", [128, 128])
    iota_t = sbuf_const("io", [128, 128])

    RG = [list(range(NCORE))]
    ccsem = nc.alloc_semaphore("ccsem")
    ccval = [0]

    def collective(kind, op, ins, outs):
        nc.gpsimd.collective_compute(
            kind, op, replica_groups=RG, ins=ins, outs=outs,
        ).then_inc(ccsem, 1)
        ccval[0] += 1
        nc.gpsimd.wait_ge(ccsem, ccval[0])
        nc.all_engine_barrier()

    def gather(pool, idx_t, col0, n, src, tag, q):
        t = pool.tile([128, (n // 128) * D], f32, tag=tag)
        nc.gpsimd.dma_gather(
            t[:].rearrange('p (b d) -> p b d', d=D), src[:, :],
            idx_t[:][:, col0 // 16:(col0 + n) // 16],
            n, n, D, single_packet=False, queue_num=q)
        return t

    def g_chunked(pool, idx_t, total, src, tag):
        out = []
        off = 0
        while off < total:
            n = min(GCAP, total - off)
            out.append((gather(pool, idx_t, off, n, src,
                               f"{tag}{len(out)}", len(out) % 4), n))
            off += n
        return out

    def blocks_of(gtiles):
        out = []
        for t, n in gtiles:
            for b in range(n // 128):
                out.append((t, b))
        return out

    def seg_reduce(pool, psum, gt, dloc_t, lo, nch_, cb, pstag):
        """One-hot seg-sum. For each chunk ch: matmuls over the full
        128-blocks its budget window touches; one-hot rows built by
        (dloc - 128*ch) == iota so out-of-chunk rows vanish."""
        for ch in range(nch_):
            lo_, hi_ = int(lo[ch]), int(lo[ch + 1])
            ps = psum.tile([128, D], f32, tag=pstag, space="PSUM")
            b0, b1_ = lo_ // 128, (hi_ + 127) // 128
            for bi, blk in enumerate(range(b0, b1_)):
                ti, off = gt[blk]
                sel = pool.tile([128, 128], f32, tag="sel")
                nc.vector.tensor_scalar(
                    out=sel[:], in0=iota_t[:], scalar1=float(128 * ch),
                    scalar2=dloc_t[:][:, blk:blk + 1], op0=ALU.add,
                    op1=ALU.is_equal)
                nc.tensor.matmul(
                    out=ps[:], lhsT=sel[:],
                    rhs=ti[:][:, off * D:(off + 1) * D],
                    start=(bi == 0), stop=(blk == b1_ - 1))
            cb(ch, ps)

    # ---------------- init: load constants + copy slice ----------------
    with PhaseTC(nc) as tc:
        with tc.tile_pool(name="ld", bufs=1):
            for name, dram in (
                    ("s1", serve1), ("h2", hop2), ("s2", send2g), ("r2", r2g),
                    ("dm1", dmod1), ("inv", invcnt), ("cf", cfg),
                    ("dmc", dmodc), ("qe", qeg), ("ke", keg), ("s3", send3g),
                    ("r3", r3g), ("dm3", dmod3), ("wqe", WQE), ("wke", WKE),
                    ("wve", WVE), ("wote", WoTe), ("bobc", bobc_d),
                    ("biasA", biasA_d), ("biasB", biasB_d), ("wq2", Wq2T),
                    ("wk2", Wk2T), ("wv2", Wv2T), ("id", ident_d),
                    ("io", iota_d)):
                nc.sync.dma_start(out=sb[name][:], in_=dram[:, :])
            nc.sync.dma_start(out=xw[:, :], in_=xsl0[:, :])

    for it in range(ITERS):
        # ---------- TC-A: serve gat1 rows ----------
        with PhaseTC(nc) as tc:
            with tc.tile_pool(name="sv", bufs=2) as pool:
                for kc in range(NCORE):
                    t = gather(pool, idx_s1, kc * B1, B1, xw, "sv", kc % 4)
                    nc.sync.dma_start(
                        out=send1[kc * B1:(kc + 1) * B1, :].rearrange(
                            "(b p) d -> p b d", p=128),
                        in_=t[:].rearrange("p (b d) -> p b d", d=D))
        collective("AllToAll", ALU.bypass, [send1.ap().opt()],
                   [recv1.ap().opt()])

        # ---------- TC-B: attention ----------
        with PhaseTC(nc) as tc:
            with tc.tile_pool(name="gt", bufs=2) as gpool, \
                 tc.tile_pool(name="st", bufs=2) as spool, \
                 tc.tile_pool(name="cl", bufs=3) as clp, \
                 tc.tile_pool(name="psb", bufs=1, space="PSUM") as psb:
                ngrp = (CPC + 46) // 47
                for grp in range(ngrp):
                    c0 = grp * 47
                    ncl = min(47, CPC - c0)
                    gt = gather(gpool, idx_h2, c0 * 256, ncl * 256, recv1,
                                "g", grp % 4)
                    stg = spool.tile([128, 94 * D], f32, tag="stg")
                    for cc in range(ncl):
                        c = c0 + cc
                        x1 = gt[:][:, (2 * cc) * D:(2 * cc + 1) * D]
                        x2 = gt[:][0:64, (2 * cc + 1) * D:(2 * cc + 2) * D]
                        t1 = psb.tile([64, 128], f32, tag="tr", space="PSUM")
                        nc.tensor.transpose(out=t1[:], in_=x1,
                                            identity=ident_t[:])
                        t2 = psb.tile([64, 128], f32, tag="tr", space="PSUM")
                        nc.tensor.transpose(out=t2[:, 0:64], in_=x2,
                                            identity=ident_t[:][0:64, 0:64])
                        xte = clp.tile([64, NNOD], f32, tag="xte")
                        nc.scalar.activation(out=xte[0:64, 0:128], in_=t1[:],
                                             func=AF.Copy)
                        nc.scalar.activation(out=xte[0:64, 128:NNOD],
                                             in_=t2[:, 0:64], func=AF.Copy)
                        qte_p = psb.tile([64, NNOD], f32, tag="qk",
                                         space="PSUM")
                        nc.tensor.matmul(out=qte_p[:], lhsT=wqe_t[:],
                                         rhs=xte[:], start=True, stop=True)
                        qte = clp.tile([64, NNOD], f32, tag="qtes")
                        nc.vector.tensor_copy(out=qte[:], in_=qte_p[:])
                        kte_p = psb.tile([64, NNOD], f32, tag="qk",
                                         space="PSUM")
                        nc.tensor.matmul(out=kte_p[:], lhsT=wke_t[:],
                                         rhs=xte[:], start=True, stop=True)
                        kte = clp.tile([64, NNOD], f32, tag="ktes")
                        nc.scalar.activation(out=kte[:], in_=kte_p[:],
                                             func=AF.Copy)
                        v1p = psb.tile([128, 64], f32, tag="v", space="PSUM")
                        nc.tensor.matmul(out=v1p[:], lhsT=xte[:, 0:128],
                                         rhs=wve_t[:], start=True, stop=True)
                        v1 = clp.tile([128, 65], f32, tag="v1s")
                        nc.vector.tensor_copy(out=v1[:, 0:64], in_=v1p[:])
                        nc.vector.memset(v1[:, 64:65], 1.0)
                        v2p = psb.tile([128, 64], f32, tag="v", space="PSUM")
                        nc.tensor.matmul(out=v2p[0:64, :],
                                         lhsT=xte[:, 128:NNOD],
                                         rhs=wve_t[:], start=True, stop=True)
                        v2 = clp.tile([64, 65], f32, tag="v2s")
                        nc.vector.tensor_copy(out=v2[:, 0:64],
                                              in_=v2p[0:64, :])
                        nc.vector.memset(v2[:, 64:65], 1.0)
                        st1p = psb.tile([128, NNOD], f32, tag="sta",
                                        space="PSUM")
                        nc.tensor.matmul(out=st1p[:], lhsT=kte[:, 0:128],
                                         rhs=qte[:], start=True, stop=True)
                        st2p = psb.tile([128, NNOD], f32, tag="stb",
                                        space="PSUM")
                        nc.tensor.matmul(out=st2p[0:64, :],
                                         lhsT=kte[:, 128:NNOD],
                                         rhs=qte[:], start=True, stop=True)
                        tb1 = clp.tile([128, NNOD], f32, tag="tb1")
                        nc.vector.tensor_scalar(
                            out=tb1[:], in0=st1p[:],
                            scalar1=biasA_t[:][:, c:c + 1], scalar2=None,
                            op0=ALU.add)
                        lr1 = clp.tile([128, NNOD], f32, tag="lr1")
                        nc.vector.tensor_scalar(
                            out=lr1[:], in0=tb1[:], scalar1=NEG,
                            scalar2=None, op0=ALU.mult)
                        nc.vector.tensor_tensor(out=lr1[:], in0=lr1[:],
                                                in1=tb1[:], op=ALU.max)
                        ex1 = clp.tile([128, NNOD], f32, tag="ex1")
                        nc.scalar.activation(out=ex1[:], in_=lr1[:],
                                             func=AF.Exp)
                        tb2 = clp.tile([64, NNOD], f32, tag="tb2")
                        nc.vector.tensor_scalar(
                            out=tb2[:], in0=st2p[0:64, :],
                            scalar1=biasB_t[:][:, c:c + 1], scalar2=None,
                            op0=ALU.add)
                        lr2 = clp.tile([64, NNOD], f32, tag="lr2")
                        nc.vector.tensor_scalar(
                            out=lr2[:], in0=tb2[:], scalar1=NEG,
                            scalar2=None, op0=ALU.mult)
                        nc.vector.tensor_tensor(out=lr2[:], in0=lr2[:],
                                                in1=tb2[:], op=ALU.max)
                        ex2 = clp.tile([64, NNOD], f32, tag="ex2")
                        nc.scalar.activation(out=ex2[:], in_=lr2[:],
                                             func=AF.Exp)
                        o1p = psb.tile([128, 65], f32, tag="o", space="PSUM")
                        nc.tensor.matmul(out=o1p[:], lhsT=ex1[:, 0:128],
                                         rhs=v1[:], start=True, stop=False)
                        nc.tensor.matmul(out=o1p[:], lhsT=ex2[:, 0:128],
                                         rhs=v2[:], start=False, stop=True)
                        o2p = psb.tile([128, 65], f32, tag="o", space="PSUM")
                        nc.tensor.matmul(out=o2p[0:64, :],
                                         lhsT=ex1[:, 128:NNOD],
                                         rhs=v1[:], start=True, stop=False)
                        nc.tensor.matmul(out=o2p[0:64, :],
                                         lhsT=ex2[:, 128:NNOD],
                                         rhs=v2[:], start=False, stop=True)
                        rq1 = clp.tile([128, 1], f32, tag="rq1")
                        nc.vector.reciprocal(out=rq1[:], in_=o1p[:, 64:65])
                        nc.vector.tensor_scalar(
                            out=stg[:, (2 * cc) * D:(2 * cc + 1) * D],
                            in0=o1p[:, 0:64], scalar1=rq1[:, 0:1],
                            scalar2=None, op0=ALU.mult)
                        rq2 = clp.tile([64, 1], f32, tag="rq2")
                        nc.vector.reciprocal(out=rq2[:],
                                             in_=o2p[0:64, 64:65])
                        nc.vector.tensor_scalar(
                            out=stg[0:64, (2 * cc + 1) * D:(2 * cc + 2) * D],
                            in0=o2p[0:64, 0:64], scalar1=rq2[:, 0:1],
                            scalar2=None, op0=ALU.mult)
                    nc.sync.dma_start(
                        out=h_buf[c0 * 256:c0 * 256 + ncl * 256, :].rearrange(
                            "(b p) d -> p b d", p=128),
                        in_=stg[:, :ncl * 2 * D].rearrange(
                            "p (b d) -> p b d", d=D))
                off = 0
                qi = 0
                while off < NCORE * B2:
                    n = min(GCAP, NCORE * B2 - off)
                    t = gather(spool, idx_s2, off, n, h_buf, "s2t", qi % 4)
                    nc.sync.dma_start(
                        out=send2[off:off + n, :].rearrange(
                            "(b p) d -> p b d", p=128),
                        in_=t[:][:, :(n // 128) * D].rearrange(
                            "p (b d) -> p b d", d=D))
                    off += n
                    qi += 1
        collective("AllToAll", ALU.bypass, [send2.ap().opt()],
                   [recv2.ap().opt()])

        # ---------- TC-C: gat1 reduce + epilogue + cf ----------
        with PhaseTC(nc) as tc:
            with tc.tile_pool(name="rg", bufs=1) as rpool, \
                 tc.tile_pool(name="wk", bufs=2) as wpool, \
                 tc.tile_pool(name="psr", bufs=2, space="PSUM") as psr:
                gt = blocks_of(g_chunked(rpool, idx_r2, NR2, recv2, "r2"))

                def epi(ch, ps):
                    tn = wpool.tile([128, D], f32, tag="tn")
                    nc.vector.tensor_scalar(
                        out=tn[:], in0=ps[:],
                        scalar1=inv_t[:][:, ch:ch + 1],
                        scalar2=None, op0=ALU.mult)
                    tt = psr.tile([64, 128], f32, tag="tt", space="PSUM")
                    nc.tensor.transpose(out=tt[:], in_=tn[:],
                                        identity=ident_t[:])
                    tte = wpool.tile([64, 128], f32, tag="tte")
                    nc.scalar.activation(out=tte[:], in_=tt[:],
                                         func=AF.Copy)
                    yp = psr.tile([128, D], f32, tag="yp", space="PSUM")
                    nc.tensor.matmul(out=yp[:], lhsT=tte[:], rhs=wote_t[:],
                                     start=True, stop=True)
                    xo = wpool.tile([128, D], f32, tag="xo")
                    nc.sync.dma_start(out=xo[:],
                                      in_=xw[ch * 128:(ch + 1) * 128, :])
                    xn = wpool.tile([128, D], f32, tag="xn")
                    nc.vector.tensor_tensor(out=xn[:], in0=yp[:], in1=xo[:],
                                            op=ALU.add)
                    nc.vector.tensor_tensor(out=xn[:], in0=xn[:],
                                            in1=bobc_t[:], op=ALU.add)
                    nc.sync.dma_start(out=xw[ch * 128:(ch + 1) * 128, :],
                                      in_=xn[:])
                seg_reduce(wpool, psr, gt, dmod1_t, LO1, NCH, epi, "seg1")

                gtc = blocks_of(g_chunked(rpool, idx_cf, NCF, xw, "cfg"))

                def cfcb(ch, ps):
                    cfs = wpool.tile([128, D], f32, tag="cfs")
                    nc.vector.tensor_copy(out=cfs[:], in_=ps[:])
                    nc.sync.dma_start(
                        out=cfpart[ch * 128:(ch + 1) * 128, :], in_=cfs[:])
                seg_reduce(wpool, psr, gtc, dmodc_t, LOC, NCC, cfcb, "seg2")
        collective("AllReduce", ALU.add, [cfpart.ap().opt()],
                   [cfall.ap().opt()])

        # ---------- TC-D: gat2 edges ----------
        with PhaseTC(nc) as tc:
            with tc.tile_pool(name="g2", bufs=2) as pool, \
                 tc.tile_pool(name="psc", bufs=2, space="PSUM") as psc:
                cft = pool.tile([64, 1024], f32, tag="cft", bufs=1)
                for j in range(8):
                    cfin = pool.tile([128, D], f32, tag="cfin")
                    nc.sync.dma_start(out=cfin[:],
                                      in_=cfall[j * 128:(j + 1) * 128, :])
                    tp = psc.tile([64, 128], f32, tag="cftp", space="PSUM")
                    nc.tensor.transpose(out=tp[:], in_=cfin[:],
                                        identity=ident_t[:])
                    nc.vector.tensor_scalar(
                        out=cft[:, j * 128:(j + 1) * 128], in0=tp[:],
                        scalar1=1.0 / VC, scalar2=None, op0=ALU.mult)
                for (wt, dst) in ((wq2_t, q2b), (wk2_t, k2b), (wv2_t, v2b)):
                    for j in range(8):
                        pp = psc.tile([128, D], f32, tag="qkv", space="PSUM")
                        nc.tensor.matmul(out=pp[:],
                                         lhsT=cft[:, j * 128:(j + 1) * 128],
                                         rhs=wt[:], start=True, stop=True)
                        ss = pool.tile([128, D], f32, tag="qkvs")
                        nc.vector.tensor_copy(out=ss[:], in_=pp[:])
                        nc.sync.dma_start(out=dst[j * 128:(j + 1) * 128, :],
                                          in_=ss[:])
                qe = gather(pool, idx_qe, 0, 1024, q2b, "qet", 0)
                ke = gather(pool, idx_ke, 0, 1024, k2b, "ket", 1)
                ve = gather(pool, idx_ke, 0, 1024, v2b, "vet", 2)
                a_t = pool.tile([128, 8], f32, tag="a")
                for j in range(8):
                    tmp = pool.tile([128, D], f32, tag="tmp")
                    nc.vector.tensor_tensor(
                        out=tmp[:], in0=qe[:][:, j * D:(j + 1) * D],
                        in1=ke[:][:, j * D:(j + 1) * D], op=ALU.mult)
                    nc.vector.tensor_reduce(
                        out=a_t[:, j:j + 1], in_=tmp[:],
                        axis=mybir.AxisListType.X, op=ALU.add)
                lr = pool.tile([128, 8], f32, tag="alr")
                nc.vector.tensor_scalar(out=lr[:], in0=a_t[:], scalar1=NEG,
                                        scalar2=None, op0=ALU.mult)
                nc.vector.tensor_tensor(out=lr[:], in0=lr[:], in1=a_t[:],
                                        op=ALU.max)
                sg = pool.tile([128, 8], f32, tag="asg")
                nc.scalar.activation(out=sg[:], in_=lr[:], func=AF.Sigmoid)
                ust = pool.tile([128, 8 * D], f32, tag="ust")
                for j in range(8):
                    nc.vector.tensor_scalar(
                        out=ust[:, j * D:(j + 1) * D],
                        in0=ve[:][:, j * D:(j + 1) * D],
                        scalar1=sg[:, j:j + 1], scalar2=None, op0=ALU.mult)
                nc.sync.dma_start(
                    out=u_buf[:, :].rearrange("(b p) d -> p b d", p=128),
                    in_=ust[:].rearrange("p (b d) -> p b d", d=D))
                off = 0
                qi = 0
                while off < NCORE * B3:
                    n = min(GCAP, NCORE * B3 - off)
                    t = gather(pool, idx_s3, off, n, u_buf, "s3t", qi % 4)
                    nc.sync.dma_start(
                        out=send3[off:off + n, :].rearrange(
                            "(b p) d -> p b d", p=128),
                        in_=t[:][:, :(n // 128) * D].rearrange(
                            "p (b d) -> p b d", d=D))
                    off += n
                    qi += 1
        collective("AllToAll", ALU.bypass, [send3.ap().opt()],
                   [recv3.ap().opt()])

        # ---------- TC-E: gat2 reduce ----------
        with PhaseTC(nc) as tc:
            with tc.tile_pool(name="r3", bufs=1) as rpool, \
                 tc.tile_pool(name="w3", bufs=2) as wpool, \
                 tc.tile_pool(name="ps3", bufs=2, space="PSUM") as ps3:
                gt3 = blocks_of(g_chunked(rpool, idx_r3, NR3, recv3, "r3"))

                def upd(ch, ps):
                    xo = wpool.tile([128, D], f32, tag="xo3")
                    nc.sync.dma_start(out=xo[:],
                                      in_=xw[ch * 128:(ch + 1) * 128, :])
                    xn = wpool.tile([128, D], f32, tag="xn3")
                    nc.vector.tensor_tensor(out=xn[:], in0=ps[:], in1=xo[:],
                                            op=ALU.add)
                    nc.sync.dma_start(out=xw[ch * 128:(ch + 1) * 128, :],
                                      in_=xn[:])
                seg_reduce(wpool, ps3, gt3, dmod3_t, LO3, NVCH, upd, "seg3")

    # ---------- readout ----------
    with PhaseTC(nc) as tc:
        with tc.tile_pool(name="ro", bufs=3) as pool, \
             tc.tile_pool(name="psm", bufs=1, space="PSUM") as psm:
            rmask_t = pool.tile([128, 2 * NCH], f32, tag="rm", bufs=1)
            nc.sync.dma_start(out=rmask_t[:], in_=rmask_d[:, :])
            sp = psm.tile([2, D], f32, tag="sums", space="PSUM")
            for ch in range(NCH):
                xt = pool.tile([128, D], f32, tag="xt")
                nc.sync.dma_start(out=xt[:],
                                  in_=xw[ch * 128:(ch + 1) * 128, :])
                nc.tensor.matmul(out=sp[:],
                                 lhsT=rmask_t[:, 2 * ch:2 * ch + 2],
                                 rhs=xt[:], start=(ch == 0),
                                 stop=(ch == NCH - 1))
            ssb = pool.tile([2, D], f32, tag="ssb")
            nc.vector.tensor_copy(out=ssb[:], in_=sp[:])
            nc.sync.dma_start(out=sums_d[:, :], in_=ssb[:])
    collective("AllReduce", ALU.add, [sums_d.ap().opt()], [sums_a.ap().opt()])
    with PhaseTC(nc) as tc:
        with tc.tile_pool(name="fin", bufs=1) as pool, \
             tc.tile_pool(name="psf", bufs=1, space="PSUM") as psf:
            sm = pool.tile([2, D], f32, tag="sm")
            nc.sync.dma_start(out=sm[:], in_=sums_a[:, :])
            sc = pool.tile([2, 1], f32, tag="sc")
            nc.sync.dma_start(out=sc[:], in_=scale2_d[:, :])
            mn = pool.tile([2, D], f32, tag="mn")
            nc.vector.tensor_scalar(out=mn[:], in0=sm[:], scalar1=sc[:, 0:1],
                                    scalar2=None, op0=ALU.mult)
            th = pool.tile([2, D], f32, tag="th")
            nc.scalar.activation(out=th[:], in_=mn[:], func=AF.Tanh)
            tp = psf.tile([64, 2], f32, tag="tp", space="PSUM")
            nc.tensor.transpose(out=tp[:, 0:2], in_=th[:],
                                identity=ident_t[:][0:2, 0:2])
            tsb = pool.tile([64, 2], f32, tag="tsb")
            nc.vector.tensor_copy(out=tsb[:], in_=tp[:, 0:2])
            pooled = pool.tile([128, 1], f32, tag="pooled")
            nc.sync.dma_start(out=pooled[0:64, :], in_=tsb[:, 0:1])
            nc.sync.dma_start(out=pooled[64:128, :], in_=tsb[:, 1:2])
            w1t = pool.tile([128, HID], f32, tag="w1t")
            nc.sync.dma_start(out=w1t[:], in_=W1T[:, :])
            b1s = pool.tile([HID, 1], f32, tag="b1s")
            nc.sync.dma_start(out=b1s[:], in_=b1t[:, :])
            hp = psf.tile([HID, 1], f32, tag="hp", space="PSUM")
            nc.tensor.matmul(out=hp[:], lhsT=w1t[:], rhs=pooled[:],
                             start=True, stop=True)
            hs = pool.tile([HID, 1], f32, tag="hs")
            nc.scalar.activation(out=hs[:], in_=hp[:], func=AF.Relu,
                                 bias=b1s[:])
            w2t = pool.tile([HID, 1], f32, tag="w2t")
            nc.sync.dma_start(out=w2t[:], in_=W2T[:, :])
            op = psf.tile([1, 1], f32, tag="op", space="PSUM")
            nc.tensor.matmul(out=op[:], lhsT=w2t[:], rhs=hs[:], start=True,
                             stop=True)
            b2s = pool.tile([1, 1], f32, tag="b2s")
            nc.sync.dma_start(out=b2s[:], in_=b2t[:, :])
            ores = pool.tile([1, 1], f32, tag="ores")
            nc.vector.tensor_tensor(out=ores[:], in0=op[:], in1=b2s[:],
                                    op=ALU.add)
            nc.sync.dma_start(out=y[:, :], in_=ores[:])

    nc.compile()
    return nc


NAMES = ["xsl0", "serve1", "hop2", "send2g", "r2g", "dmod1", "invcnt",
         "cfg", "dmodc", "qeg", "keg", "send3g", "r3g", "dmod3", "biasA",
         "biasB", "bo_bc", "WQE", "WKE", "WVE", "WoTe", "Wq2T", "Wk2T",
         "Wv2T", "W1T", "b1", "W2T", "b2", "ident", "iota", "scale2",
         "rmask"]


def kernel(**inputs):
    from concourse.bass_utils import run_bass_kernel_spmd
    dims, per = _plan(inputs)
    nc = _build(dims)
    in_maps = [{n: np.ascontiguousarray(per[k][n]) for n in NAMES}
               for k in range(NCORE)]
    res = run_bass_kernel_spmd(nc, in_maps, list(range(NCORE)))
    return np.asarray(res.results[0]["y"]).reshape(1).astype(np.float32)



# revision 12
# speedup vs baseline: 1.2370x; 1.2370x over previous
"""AttnJGNN Trainium2 kernel: 8-core SPMD, pull-model sharded tables.

Structure per iteration (x-table chunk-interleaved across 8 cores):
  TC-A  owners serve gat1 node rows (dma_gather from slice) -> AllToAll
  TC-B  per-cluster attention (transposed-score formulation) -> h rows,
        re-gather into dest-owner buckets -> AllToAll
  TC-C  dest-sorted one-hot seg-sum matmuls + gat1 epilogue (invcnt, Wo, bo,
        residual) writes slice; cf partial sums by cluster -> AllReduce
  TC-D  QKV on cf, per-edge attention, expansion to shared vars -> AllToAll
  TC-E  one-hot seg-sum into x_var slice chunks
Readout: masked column-sum matmuls -> AllReduce -> tanh/MLP on every core.
"""
import numpy as np

N = 50000
M = 100000
C = 1000
VC = 64
KC = 128
E = 8000
S = 8
D = 64
HID = 64
NEG = 0.2
GAMMA = 1.0
ITERS = 3
ACTIVE = 4

NCORE = 8
ROWS = N + M                      # 150000
NCHG = (ROWS + 127) // 128        # 1172 global chunks
TPAD = NCHG * 128                 # 150016
NCH = (NCHG + NCORE - 1) // NCORE  # 147 chunks per core (padded)
SL = NCH * 128                    # 18816 slice rows
CPC = C // NCORE                  # 125 clusters per core
EPC = E // NCORE                  # 1000 edges per core
NNOD = VC + KC                    # 192 nodes per cluster
HB = CPC * 256                    # 32000 h_buf rows
GCAP = 12032                      # dma_gather per-call index cap


def _r128(x):
    return (int(x) + 127) // 128 * 128


def _r16(x):
    return (int(x) + 15) // 16 * 16


def _pack16(idx, width=None):
    """Pack flat int index list -> [128, ceil(n/16)] int16 (16-wrapped,
    replicated across the 8 16-partition groups)."""
    idx = np.asarray(idx)
    n = len(idx)
    cols = (n + 15) // 16
    if width is None:
        width = cols
    buf = np.zeros(cols * 16, np.int16)
    buf[:n] = idx.astype(np.int16)
    arr = np.zeros((128, width), np.int16)
    w = buf.reshape(cols, 16).T
    for g in range(8):
        arr[g * 16:(g + 1) * 16, :cols] = w
    return arr


def _owner(r):
    return (r >> 7) % NCORE


def _local(r):
    return ((r >> 7) // NCORE) * 128 + (r & 127)


def _plan(inp):
    """All host-side planning. Returns (dims, per_core list of dicts)."""
    xv = np.asarray(inp['x_var'], np.float32)
    xc = np.asarray(inp['x_clause'], np.float32)
    sat = np.asarray(inp['satisfaction_scores'], np.float32)
    cvar = np.asarray(inp['cluster_var_ids']).astype(np.int64)
    ccla = np.asarray(inp['cluster_clause_ids']).astype(np.int64)
    ei = np.asarray(inp['cluster_edge_index']).astype(np.int64)
    sv = np.asarray(inp['shared_vars']).astype(np.int64)

    xall = np.zeros((TPAD, D), np.float32)
    xall[:N] = xv
    xall[N:ROWS] = xc

    nodes = np.concatenate([cvar, ccla + N], axis=1)      # [C, 192]

    dims = {}
    per = [dict() for _ in range(NCORE)]

    # ---- x slices ----
    for k in range(NCORE):
        sl = np.zeros((SL, D), np.float32)
        for p in range(NCH):
            ch = p * NCORE + k
            if ch < NCHG:
                sl[p * 128:(p + 1) * 128] = xall[ch * 128:(ch + 1) * 128]
        per[k]['xsl0'] = sl

    # ---- gat1 serve (owners -> consumers) ----
    # L1[o][k]: local rows in consumer-k scan order
    L1 = [[None] * NCORE for _ in range(NCORE)]
    for k in range(NCORE):
        flat = nodes[k * CPC:(k + 1) * CPC].reshape(-1)   # [24000] scan order
        o = _owner(flat)
        lo = _local(flat)
        for j in range(NCORE):
            L1[j][k] = lo[o == j]
    B1 = _r128(max(len(L1[j][k]) for j in range(NCORE) for k in range(NCORE)))
    dims['B1'] = B1
    for j in range(NCORE):
        idx = np.zeros(NCORE * B1, np.int64)
        for k in range(NCORE):
            idx[k * B1:k * B1 + len(L1[j][k])] = L1[j][k]
        per[j]['serve1'] = _pack16(idx)

    # ---- gat1 hop2 (consumer gathers cluster tiles from recv1) ----
    # position g = cl*256 + r (r<192 real); recv1 row = o*B1 + rank
    for k in range(NCORE):
        flat = nodes[k * CPC:(k + 1) * CPC].reshape(-1)
        o = _owner(flat)
        g2 = np.zeros(CPC * 256, np.int64)
        cnt = np.zeros(NCORE, np.int64)
        ranks = np.zeros(len(flat), np.int64)
        for j in range(NCORE):
            m = o == j
            ranks[m] = j * B1 + np.arange(m.sum())
        g2r = ranks.reshape(CPC, NNOD)
        g2 = np.zeros((CPC, 256), np.int64)
        g2[:, :NNOD] = g2r
        per[k]['hop2'] = _pack16(g2.reshape(-1))

    # ---- gat1 contributions: h_buf -> dest-owner buckets ----
    L2len = np.zeros((NCORE, NCORE), np.int64)
    L2 = [[None] * NCORE for _ in range(NCORE)]   # [k][o2] -> (hrows, dloc)
    for k in range(NCORE):
        flat = nodes[k * CPC:(k + 1) * CPC].reshape(-1)
        o2 = _owner(flat)
        dloc = _local(flat)
        g = (np.arange(CPC)[:, None] * 256 + np.arange(NNOD)[None, :]).reshape(-1)
        for j in range(NCORE):
            m = o2 == j
            L2[k][j] = (g[m], dloc[m])
            L2len[k, j] = m.sum()
    B2 = _r128(L2len.max())
    dims['B2'] = B2
    for k in range(NCORE):
        idx = np.zeros(NCORE * B2, np.int64)
        for j in range(NCORE):
            idx[j * B2:j * B2 + len(L2[k][j][0])] = L2[k][j][0]
        per[k]['send2g'] = _pack16(idx)

    # ---- gat1 reduce plan (owner side) ----
    # entries: (dest_local, recv2row = src*B2 + i), sorted by dest
    cnt1 = np.zeros((NCORE, NCH), np.int64)
    ent = []
    for j in range(NCORE):
        ds, rs = [], []
        for s_ in range(NCORE):
            g, dl = L2[s_][j]
            ds.append(dl)
            rs.append(s_ * B2 + np.arange(len(g)))
        ds = np.concatenate(ds)
        rs = np.concatenate(rs)
        srt = np.argsort(ds, kind='stable')
        ds, rs = ds[srt], rs[srt]
        ent.append((ds, rs))
        np.add.at(cnt1[j], ds >> 7, 1)
    BUD1 = np.maximum(_r16(0) + 16, (np.ceil(cnt1.max(0) / 16) * 16).astype(np.int64))
    tot1 = int(BUD1.sum())
    tot1p = _r128(tot1)
    dims['NR2'] = tot1p
    lo1 = np.zeros(NCH + 1, np.int64)
    lo1[1:] = np.cumsum(BUD1)
    dims['LO1'] = lo1
    for j in range(NCORE):
        ds, rs = ent[j]
        gidx = np.zeros(tot1p, np.int64)
        dmod = np.full(tot1p, -1.0, np.float32)
        chs = (ds >> 7).astype(np.int64)
        for ch in range(NCH):
            m = chs == ch
            nn_ = int(m.sum())
            gidx[lo1[ch]:lo1[ch] + nn_] = rs[m]
            dmod[lo1[ch]:lo1[ch] + nn_] = ds[m].astype(np.float32)
        per[j]['r2g'] = _pack16(gidx)
        per[j]['dmod1'] = dmod.reshape(-1, 128).T.copy()   # [128, tot1p/128]
        # invcnt
        cntfull = np.zeros(SL, np.float32)
        np.add.at(cntfull, ent[j][0], 1.0)
        inv = 1.0 / np.maximum(cntfull, 1.0)
        per[j]['invcnt'] = inv.reshape(NCH, 128).T.copy()  # [128, NCH]

    # ---- cf plan: member instances by owner, seg-sum by cluster ----
    NCC = (C + 127) // 128                                 # 8 cluster chunks
    mrow = cvar.reshape(-1)                                # [64000] scan (c,m)
    mc = np.repeat(np.arange(C), VC)
    mo = _owner(mrow)
    ml = _local(mrow)
    cntc = np.zeros((NCORE, NCC), np.int64)
    cfent = []
    for j in range(NCORE):
        m = mo == j
        cs, ls = mc[m], ml[m]                              # sorted by c already
        cfent.append((cs, ls))
        np.add.at(cntc[j], cs >> 7, 1)
    BUDC = (np.ceil(cntc.max(0) / 16) * 16).astype(np.int64) + 16
    totc = int(BUDC.sum())
    totcp = _r128(totc)
    dims['NCF'] = totcp
    loc_ = np.zeros(NCC + 1, np.int64)
    loc_[1:] = np.cumsum(BUDC)
    dims['LOC'] = loc_
    dims['NCC'] = NCC
    for j in range(NCORE):
        cs, ls = cfent[j]
        gidx = np.zeros(totcp, np.int64)
        dmod = np.full(totcp, -1.0, np.float32)
        chs = cs >> 7
        for ch in range(NCC):
            m = chs == ch
            nn_ = int(m.sum())
            gidx[loc_[ch]:loc_[ch] + nn_] = ls[m]
            dmod[loc_[ch]:loc_[ch] + nn_] = cs[m].astype(np.float32)
        per[j]['cfg'] = _pack16(gidx)
        per[j]['dmodc'] = dmod.reshape(-1, 128).T.copy()

    # ---- gat2 edges ----
    for k in range(NCORE):
        c1 = ei[0, k * EPC:(k + 1) * EPC]
        c2 = ei[1, k * EPC:(k + 1) * EPC]
        per[k]['qeg'] = _pack16(np.concatenate([c1, np.zeros(24, np.int64)]))
        per[k]['keg'] = _pack16(np.concatenate([c2, np.zeros(24, np.int64)]))

    # expansion: pairs (e_local, s) -> dest var row
    L3len = np.zeros((NCORE, NCORE), np.int64)
    L3 = [[None] * NCORE for _ in range(NCORE)]
    for k in range(NCORE):
        dsts = sv[k * EPC:(k + 1) * EPC].reshape(-1)       # [8000]
        el = np.repeat(np.arange(EPC), S)
        o3 = _owner(dsts)
        dl = _local(dsts)
        for j in range(NCORE):
            m = o3 == j
            L3[k][j] = (el[m], dl[m])
            L3len[k, j] = m.sum()
    B3 = _r128(L3len.max())
    dims['B3'] = B3
    for k in range(NCORE):
        idx = np.zeros(NCORE * B3, np.int64)
        for j in range(NCORE):
            idx[j * B3:j * B3 + len(L3[k][j][0])] = L3[k][j][0]
        per[k]['send3g'] = _pack16(idx)

    # gat2 reduce plan: var chunks only
    NVCH = 0
    for k in range(NCORE):
        nv = sum(1 for p in range(NCH) if (p * NCORE + k) * 128 < N)
        NVCH = max(NVCH, nv)
    dims['NVCH'] = NVCH
    cnt3 = np.zeros((NCORE, NVCH), np.int64)
    ent3 = []
    for j in range(NCORE):
        ds, rs = [], []
        for s_ in range(NCORE):
            el, dl = L3[s_][j]
            ds.append(dl)
            rs.append(s_ * B3 + np.arange(len(el)))
        ds = np.concatenate(ds)
        rs = np.concatenate(rs)
        srt = np.argsort(ds, kind='stable')
        ds, rs = ds[srt], rs[srt]
        ent3.append((ds, rs))
        np.add.at(cnt3[j], ds >> 7, 1)
    BUD3 = (np.ceil(cnt3.max(0) / 16) * 16).astype(np.int64) + 16
    tot3 = int(BUD3.sum())
    tot3p = _r128(tot3)
    dims['NR3'] = tot3p
    lo3 = np.zeros(NVCH + 1, np.int64)
    lo3[1:] = np.cumsum(BUD3)
    dims['LO3'] = lo3
    for j in range(NCORE):
        ds, rs = ent3[j]
        gidx = np.zeros(tot3p, np.int64)
        dmod = np.full(tot3p, -1.0, np.float32)
        chs = ds >> 7
        for ch in range(NVCH):
            m = chs == ch
            nn_ = int(m.sum())
            gidx[lo3[ch]:lo3[ch] + nn_] = rs[m]
            dmod[lo3[ch]:lo3[ch] + nn_] = ds[m].astype(np.float32)
        per[j]['r3g'] = _pack16(gidx)
        per[j]['dmod3'] = dmod.reshape(-1, 128).T.copy()

    # ---- per-cluster bias columns (key-partition layout) ----
    for k in range(NCORE):
        b = np.zeros((CPC, NNOD), np.float32)
        b[:, VC:] = GAMMA * sat[ccla[k * CPC:(k + 1) * CPC]]
        per[k]['biasA'] = b[:, :128].T.copy()      # [128, CPC]
        per[k]['biasB'] = b[:, 128:].T.copy()      # [64, CPC]
        per[k]['biasA2'] = (NEG * b[:, :128].T).copy()
        per[k]['biasB2'] = (NEG * b[:, 128:].T).copy()

    # ---- weights / consts (shared) ----
    hw1m = float(np.asarray(inp['hw1'])[:ACTIVE].mean())
    hw2m = float(np.asarray(inp['hw2'])[:ACTIVE].mean())
    WQ1 = np.asarray(inp['WQ1'], np.float32)
    WK1 = np.asarray(inp['WK1'], np.float32)
    WV1 = np.asarray(inp['WV1'], np.float32)
    Wo = np.asarray(inp['Wo'], np.float32)
    bo = np.asarray(inp['bo'], np.float32)
    WQE = (WQ1.T / np.sqrt(D)).astype(np.float32)
    WKE = WK1.T.astype(np.float32).copy()
    # Wo folded into the value projection: h' = h @ Wo.T distributes over
    # the scatter-sum and invcnt division, so the epilogue needs no matmul.
    WVE = ((Wo @ WV1).T * hw1m).astype(np.float32)
    WoTe = Wo.T.astype(np.float32).copy()
    bo_bc = np.tile(bo[None, :], (128, 1)).astype(np.float32)
    shared = {
        'WQE': WQE, 'WKE': WKE, 'WVE': WVE, 'WoTe': WoTe, 'bo_bc': bo_bc,
        'Wq2T': (np.asarray(inp['WQ2'], np.float32).T / np.sqrt(D)).copy(),
        'Wk2T': np.asarray(inp['WK2'], np.float32).T.copy(),
        'Wv2T': (np.asarray(inp['WV2'], np.float32).T * hw2m).copy(),
        'W1T': np.asarray(inp['W1'], np.float32).T.copy(),      # [128, 64]
        'b1': np.asarray(inp['b1'], np.float32).reshape(HID, 1).copy(),
        'W2T': np.asarray(inp['W2'], np.float32).T.copy(),      # [64, 1]
        'b2': np.asarray(inp['b2'], np.float32).reshape(1, 1).copy(),
        'ident': np.eye(128, dtype=np.float32),
        'iota': np.tile(np.arange(128, dtype=np.float32)[None, :], (128, 1)),
        'scale2': np.array([[1.0 / N], [1.0 / M]], np.float32),
    }

    # ---- readout masks [128, 2*NCH] ----
    for k in range(NCORE):
        mask = np.zeros((128, 2 * NCH), np.float32)
        for p in range(NCH):
            ch = p * NCORE + k
            if ch >= NCHG:
                continue
            rows = np.arange(ch * 128, ch * 128 + 128)
            mask[:, 2 * p] = (rows < N).astype(np.float32)
            mask[:, 2 * p + 1] = ((rows >= N) & (rows < ROWS)).astype(np.float32)
        per[k]['rmask'] = mask

    for k in range(NCORE):
        for key in shared:
            per[k][key] = shared[key]
    return dims, per


def _build(dims):
    import concourse.bass as bass
    import concourse.bacc as bacc
    import concourse.tile as tile
    from concourse import mybir
    from concourse.vector_clock import ScopedClock
    from concourse._compat import cdiv

    f32 = mybir.dt.float32
    i16 = mybir.dt.int16
    AF = mybir.ActivationFunctionType
    ALU = mybir.AluOpType

    B1, B2, B3 = dims['B1'], dims['B2'], dims['B3']
    NR2, NR3, NCF = dims['NR2'], dims['NR3'], dims['NCF']
    LO1, LO3, LOC = dims['LO1'], dims['LO3'], dims['LOC']
    NVCH, NCC = dims['NVCH'], dims['NCC']

    class PhaseTC(tile.TileContext):
        # tail without the DMA-ring-resetting drain (it breaks later
        # collectives); split multi-waits (walrus allows 1 per inst).
        def _drain_and_barrier(self, tick_clock, wait_clock):
            probe = self.nc.sync.nop(nofuse=True)
            wait_clock.add_sem_waits(
                probe.ins, ScopedClock({None: tick_clock.global_clock}))
            waits = list(probe.ins.sync_info.on_wait or [])
            if len(waits) > 1:
                probe.ins.sync_info.on_wait = waits[:1]
                for w in waits[1:]:
                    extra = self.nc.sync.nop(nofuse=True)
                    si = extra.ins.sync_info
                    if si is None:
                        si = mybir.SyncInfo(on_wait=[], on_update=[])
                    si.on_wait = [w]
                    extra.ins.sync_info = si
            self.nc.all_engine_barrier()
            assert self.sems is not None
            popped = self.nc._tile_sem_poison_stack.pop()
            assert popped is self._sem_poison
            self.nc.clear_and_free_semaphores(
                list(self.sems.allocated().values()))
            self.nc.all_engine_barrier()

    nc = bacc.Bacc("TRN2", num_swdge_queues=4)

    def din(name, shape, dtype=f32):
        return nc.dram_tensor(name, list(shape), dtype, kind="ExternalInput")

    xsl0 = din("xsl0", [SL, D])
    serve1 = din("serve1", [128, cdiv(NCORE * B1, 16)], i16)
    hop2 = din("hop2", [128, cdiv(CPC * 256, 16)], i16)
    send2g = din("send2g", [128, cdiv(NCORE * B2, 16)], i16)
    r2g = din("r2g", [128, cdiv(NR2, 16)], i16)
    dmod1 = din("dmod1", [128, NR2 // 128])
    invcnt = din("invcnt", [128, NCH])
    cfg = din("cfg", [128, cdiv(NCF, 16)], i16)
    dmodc = din("dmodc", [128, NCF // 128])
    qeg = din("qeg", [128, 64], i16)
    keg = din("keg", [128, 64], i16)
    send3g = din("send3g", [128, cdiv(NCORE * B3, 16)], i16)
    r3g = din("r3g", [128, cdiv(NR3, 16)], i16)
    dmod3 = din("dmod3", [128, NR3 // 128])
    biasA_d = din("biasA", [128, CPC])
    biasB_d = din("biasB", [64, CPC])
    biasA2_d = din("biasA2", [128, CPC])
    biasB2_d = din("biasB2", [64, CPC])
    WQE = din("WQE", [64, 64])
    WKE = din("WKE", [64, 64])
    WVE = din("WVE", [64, 64])
    WoTe = din("WoTe", [64, 64])
    bobc_d = din("bo_bc", [128, 64])
    Wq2T = din("Wq2T", [64, 64])
    Wk2T = din("Wk2T", [64, 64])
    Wv2T = din("Wv2T", [64, 64])
    W1T = din("W1T", [128, HID])
    b1t = din("b1", [HID, 1])
    W2T = din("W2T", [HID, 1])
    b2t = din("b2", [1, 1])
    ident_d = din("ident", [128, 128])
    iota_d = din("iota", [128, 128])
    scale2_d = din("scale2", [2, 1])
    rmask_d = din("rmask", [128, 2 * NCH])

    y = nc.dram_tensor("y", [1, 1], f32, kind="ExternalOutput")

    xw = nc.dram_tensor("xw", [SL, D], f32)
    send1 = nc.dram_tensor("send1", [NCORE * B1, D], f32)
    recv1 = nc.dram_tensor("recv1", [NCORE * B1, D], f32)
    h_buf = nc.dram_tensor("h_buf", [HB, D], f32)
    send2 = nc.dram_tensor("send2", [NCORE * B2, D], f32)
    recv2 = nc.dram_tensor("recv2", [NCORE * B2, D], f32)
    cfpart = nc.dram_tensor("cfpart", [1024, D], f32)
    cfall = nc.dram_tensor("cfall", [1024, D], f32)
    q2b = nc.dram_tensor("q2b", [1024, D], f32)
    k2b = nc.dram_tensor("k2b", [1024, D], f32)
    v2b = nc.dram_tensor("v2b", [1024, D], f32)
    u_buf = nc.dram_tensor("u_buf", [1024, D], f32)
    send3 = nc.dram_tensor("send3", [NCORE * B3, D], f32)
    recv3 = nc.dram_tensor("recv3", [NCORE * B3, D], f32)
    sums_d = nc.dram_tensor("sums_d", [2, D], f32)
    sums_a = nc.dram_tensor("sums_a", [2, D], f32)

    # persistent SBUF constants (outside Tile pools)
    sb = {}
    def sbuf_const(name, shape, dtype=f32):
        sb[name] = nc.alloc_sbuf_tensor(f"c_{name}", list(shape), dtype)
        return sb[name]
    idx_s1 = sbuf_const("s1", [128, cdiv(NCORE * B1, 16)], i16)
    idx_h2 = sbuf_const("h2", [128, cdiv(CPC * 256, 16)], i16)
    idx_s2 = sbuf_const("s2", [128, cdiv(NCORE * B2, 16)], i16)
    idx_r2 = sbuf_const("r2", [128, cdiv(NR2, 16)], i16)
    dmod1_t = sbuf_const("dm1", [128, NR2 // 128])
    inv_t = sbuf_const("inv", [128, NCH])
    idx_cf = sbuf_const("cf", [128, cdiv(NCF, 16)], i16)
    dmodc_t = sbuf_const("dmc", [128, NCF // 128])
    idx_qe = sbuf_const("qe", [128, 64], i16)
    idx_ke = sbuf_const("ke", [128, 64], i16)
    idx_s3 = sbuf_const("s3", [128, cdiv(NCORE * B3, 16)], i16)
    idx_r3 = sbuf_const("r3", [128, cdiv(NR3, 16)], i16)
    dmod3_t = sbuf_const("dm3", [128, NR3 // 128])
    wqe_t = sbuf_const("wqe", [64, 64])
    wke_t = sbuf_const("wke", [64, 64])
    wve_t = sbuf_const("wve", [64, 64])
    wote_t = sbuf_const("wote", [64, 64])
    bobc_t = sbuf_const("bobc", [128, 64])
    biasA_t = sbuf_const("biasA", [128, CPC])
    biasB_t = sbuf_const("biasB", [64, CPC])
    biasA2_t = sbuf_const("biasA2", [128, CPC])
    biasB2_t = sbuf_const("biasB2", [64, CPC])
    wq2_t = sbuf_const("wq2", [64, 64])
    wk2_t = sbuf_const("wk2", [64, 64])
    wv2_t = sbuf_const("wv2", [64, 64])
    ident_t = sbuf_const("id", [128, 128])
    iota_t = sbuf_const("io", [128, 128])

    RG = [list(range(NCORE))]
    ccsem = nc.alloc_semaphore("ccsem")
    ccval = [0]

    def collective(kind, op, ins, outs):
        nc.gpsimd.collective_compute(
            kind, op, replica_groups=RG, ins=ins, outs=outs,
        ).then_inc(ccsem, 1)
        ccval[0] += 1
        nc.gpsimd.wait_ge(ccsem, ccval[0])
        nc.all_engine_barrier()

    def gather(pool, idx_t, col0, n, src, tag, q):
        t = pool.tile([128, (n // 128) * D], f32, tag=tag)
        nc.gpsimd.dma_gather(
            t[:].rearrange('p (b d) -> p b d', d=D), src[:, :],
            idx_t[:][:, col0 // 16:(col0 + n) // 16],
            n, n, D, single_packet=False, queue_num=q)
        return t

    def g_chunked(pool, idx_t, total, src, tag):
        out = []
        off = 0
        while off < total:
            n = min(GCAP, total - off)
            out.append((gather(pool, idx_t, off, n, src,
                               f"{tag}{len(out)}", len(out) % 4), n))
            off += n
        return out

    def blocks_of(gtiles):
        out = []
        for t, n in gtiles:
            for b in range(n // 128):
                out.append((t, b))
        return out

    def seg_reduce(pool, psum, gt, dloc_t, lo, nch_, cb, pstag):
        """One-hot seg-sum. For each chunk ch: matmuls over the full
        128-blocks its budget window touches; one-hot rows built by
        (dloc - 128*ch) == iota so out-of-chunk rows vanish."""
        for ch in range(nch_):
            lo_, hi_ = int(lo[ch]), int(lo[ch + 1])
            ps = psum.tile([128, D], f32, tag=pstag, space="PSUM")
            b0, b1_ = lo_ // 128, (hi_ + 127) // 128
            for bi, blk in enumerate(range(b0, b1_)):
                ti, off = gt[blk]
                sel = pool.tile([128, 128], f32, tag="sel")
                nc.vector.tensor_scalar(
                    out=sel[:], in0=iota_t[:], scalar1=float(128 * ch),
                    scalar2=dloc_t[:][:, blk:blk + 1], op0=ALU.add,
                    op1=ALU.is_equal)
                nc.tensor.matmul(
                    out=ps[:], lhsT=sel[:],
                    rhs=ti[:][:, off * D:(off + 1) * D],
                    start=(bi == 0), stop=(blk == b1_ - 1))
            cb(ch, ps)

    # ---------------- init: load constants + copy slice ----------------
    with PhaseTC(nc) as tc:
        with tc.tile_pool(name="ld", bufs=1):
            for name, dram in (
                    ("s1", serve1), ("h2", hop2), ("s2", send2g), ("r2", r2g),
                    ("dm1", dmod1), ("inv", invcnt), ("cf", cfg),
                    ("dmc", dmodc), ("qe", qeg), ("ke", keg), ("s3", send3g),
                    ("r3", r3g), ("dm3", dmod3), ("wqe", WQE), ("wke", WKE),
                    ("wve", WVE), ("wote", WoTe), ("bobc", bobc_d),
                    ("biasA", biasA_d), ("biasB", biasB_d),
                    ("biasA2", biasA2_d), ("biasB2", biasB2_d),
                    ("wq2", Wq2T),
                    ("wk2", Wk2T), ("wv2", Wv2T), ("id", ident_d),
                    ("io", iota_d)):
                nc.sync.dma_start(out=sb[name][:], in_=dram[:, :])
            nc.sync.dma_start(out=xw[:, :], in_=xsl0[:, :])

    for it in range(ITERS):
        # ---------- TC-A: serve gat1 rows ----------
        with PhaseTC(nc) as tc:
            with tc.tile_pool(name="sv", bufs=2) as pool:
                for kc in range(NCORE):
                    t = gather(pool, idx_s1, kc * B1, B1, xw, "sv", kc % 4)
                    nc.sync.dma_start(
                        out=send1[kc * B1:(kc + 1) * B1, :].rearrange(
                            "(b p) d -> p b d", p=128),
                        in_=t[:].rearrange("p (b d) -> p b d", d=D))
        collective("AllToAll", ALU.bypass, [send1.ap().opt()],
                   [recv1.ap().opt()])

        # ---------- TC-B: attention ----------
        with PhaseTC(nc) as tc:
            with tc.tile_pool(name="gt", bufs=2) as gpool, \
                 tc.tile_pool(name="st", bufs=2) as spool, \
                 tc.tile_pool(name="cl", bufs=3) as clp, \
                 tc.tile_pool(name="psb", bufs=1, space="PSUM") as psb:
                ngrp = (CPC + 46) // 47
                for grp in range(ngrp):
                    c0 = grp * 47
                    ncl = min(47, CPC - c0)
                    gt = gather(gpool, idx_h2, c0 * 256, ncl * 256, recv1,
                                "g", grp % 4)
                    stg = spool.tile([128, 94 * D], f32, tag="stg")
                    for cc in range(ncl):
                        c = c0 + cc
                        x1 = gt[:][:, (2 * cc) * D:(2 * cc + 1) * D]
                        x2 = gt[:][0:64, (2 * cc + 1) * D:(2 * cc + 2) * D]
                        t1 = psb.tile([64, 128], f32, tag="tr", space="PSUM")
                        nc.tensor.transpose(out=t1[:], in_=x1,
                                            identity=ident_t[:])
                        t2 = psb.tile([64, 128], f32, tag="tr", space="PSUM")
                        nc.tensor.transpose(out=t2[:, 0:64], in_=x2,
                                            identity=ident_t[:][0:64, 0:64])
                        xte = clp.tile([64, NNOD], f32, tag="xte")
                        nc.scalar.activation(out=xte[0:64, 0:128], in_=t1[:],
                                             func=AF.Copy)
                        nc.scalar.activation(out=xte[0:64, 128:NNOD],
                                             in_=t2[:, 0:64], func=AF.Copy)
                        qte_p = psb.tile([64, NNOD], f32, tag="qk",
                                         space="PSUM")
                        nc.tensor.matmul(out=qte_p[:], lhsT=wqe_t[:],
                                         rhs=xte[:], start=True, stop=True)
                        qte = clp.tile([64, NNOD], f32, tag="qtes")
                        nc.vector.tensor_copy(out=qte[:], in_=qte_p[:])
                        kte_p = psb.tile([64, NNOD], f32, tag="qk",
                                         space="PSUM")
                        nc.tensor.matmul(out=kte_p[:], lhsT=wke_t[:],
                                         rhs=xte[:], start=True, stop=True)
                        kte = clp.tile([64, NNOD], f32, tag="ktes")
                        nc.scalar.activation(out=kte[:], in_=kte_p[:],
                                             func=AF.Copy)
                        v1p = psb.tile([128, 64], f32, tag="v", space="PSUM")
                        nc.tensor.matmul(out=v1p[:], lhsT=xte[:, 0:128],
                                         rhs=wve_t[:], start=True, stop=True)
                        v1 = clp.tile([128, 65], f32, tag="v1s")
                        nc.vector.tensor_copy(out=v1[:, 0:64], in_=v1p[:])
                        nc.vector.memset(v1[:, 64:65], 1.0)
                        v2p = psb.tile([128, 64], f32, tag="v", space="PSUM")
                        nc.tensor.matmul(out=v2p[0:64, :],
                                         lhsT=xte[:, 128:NNOD],
                                         rhs=wve_t[:], start=True, stop=True)
                        v2 = clp.tile([64, 65], f32, tag="v2s")
                        nc.vector.tensor_copy(out=v2[:, 0:64],
                                              in_=v2p[0:64, :])
                        nc.vector.memset(v2[:, 64:65], 1.0)
                        st1p = psb.tile([128, NNOD], f32, tag="sta",
                                        space="PSUM")
                        nc.tensor.matmul(out=st1p[:], lhsT=kte[:, 0:128],
                                         rhs=qte[:], start=True, stop=True)
                        st2p = psb.tile([128, NNOD], f32, tag="stb",
                                        space="PSUM")
                        nc.tensor.matmul(out=st2p[0:64, :],
                                         lhsT=kte[:, 128:NNOD],
                                         rhs=qte[:], start=True, stop=True)
                        e1a = clp.tile([128, NNOD], f32, tag="tb1")
                        nc.scalar.activation(
                            out=e1a[:], in_=st1p[:], func=AF.Exp,
                            bias=biasA_t[:][:, c:c + 1])
                        e2a = clp.tile([128, NNOD], f32, tag="lr1")
                        nc.scalar.activation(
                            out=e2a[:], in_=st1p[:], func=AF.Exp,
                            bias=biasA2_t[:][:, c:c + 1], scale=NEG)
                        ex1 = clp.tile([128, NNOD], f32, tag="ex1")
                        nc.vector.tensor_tensor(out=ex1[:], in0=e1a[:],
                                                in1=e2a[:], op=ALU.max)
                        e1b = clp.tile([64, NNOD], f32, tag="tb2")
                        nc.scalar.activation(
                            out=e1b[:], in_=st2p[0:64, :], func=AF.Exp,
                            bias=biasB_t[:][:, c:c + 1])
                        e2b = clp.tile([64, NNOD], f32, tag="lr2")
                        nc.scalar.activation(
                            out=e2b[:], in_=st2p[0:64, :], func=AF.Exp,
                            bias=biasB2_t[:][:, c:c + 1], scale=NEG)
                        ex2 = clp.tile([64, NNOD], f32, tag="ex2")
                        nc.vector.tensor_tensor(out=ex2[:], in0=e1b[:],
                                                in1=e2b[:], op=ALU.max)
                        o1p = psb.tile([128, 65], f32, tag="o", space="PSUM")
                        nc.tensor.matmul(out=o1p[:], lhsT=ex1[:, 0:128],
                                         rhs=v1[:], start=True, stop=False)
                        nc.tensor.matmul(out=o1p[:], lhsT=ex2[:, 0:128],
                                         rhs=v2[:], start=False, stop=True)
                        o2p = psb.tile([128, 65], f32, tag="o", space="PSUM")
                        nc.tensor.matmul(out=o2p[0:64, :],
                                         lhsT=ex1[:, 128:NNOD],
                                         rhs=v1[:], start=True, stop=False)
                        nc.tensor.matmul(out=o2p[0:64, :],
                                         lhsT=ex2[:, 128:NNOD],
                                         rhs=v2[:], start=False, stop=True)
                        rq1 = clp.tile([128, 1], f32, tag="rq1")
                        nc.vector.reciprocal(out=rq1[:], in_=o1p[:, 64:65])
                        nc.vector.tensor_scalar(
                            out=stg[:, (2 * cc) * D:(2 * cc + 1) * D],
                            in0=o1p[:, 0:64], scalar1=rq1[:, 0:1],
                            scalar2=None, op0=ALU.mult)
                        rq2 = clp.tile([64, 1], f32, tag="rq2")
                        nc.vector.reciprocal(out=rq2[:],
                                             in_=o2p[0:64, 64:65])
                        nc.vector.tensor_scalar(
                            out=stg[0:64, (2 * cc + 1) * D:(2 * cc + 2) * D],
                            in0=o2p[0:64, 0:64], scalar1=rq2[:, 0:1],
                            scalar2=None, op0=ALU.mult)
                    nc.sync.dma_start(
                        out=h_buf[c0 * 256:c0 * 256 + ncl * 256, :].rearrange(
                            "(b p) d -> p b d", p=128),
                        in_=stg[:, :ncl * 2 * D].rearrange(
                            "p (b d) -> p b d", d=D))
                off = 0
                qi = 0
                while off < NCORE * B2:
                    n = min(GCAP, NCORE * B2 - off)
                    t = gather(spool, idx_s2, off, n, h_buf, "s2t", qi % 4)
                    nc.sync.dma_start(
                        out=send2[off:off + n, :].rearrange(
                            "(b p) d -> p b d", p=128),
                        in_=t[:][:, :(n // 128) * D].rearrange(
                            "p (b d) -> p b d", d=D))
                    off += n
                    qi += 1
        collective("AllToAll", ALU.bypass, [send2.ap().opt()],
                   [recv2.ap().opt()])

        # ---------- TC-C: gat1 reduce + epilogue + cf ----------
        with PhaseTC(nc) as tc:
            with tc.tile_pool(name="rg", bufs=1) as rpool, \
                 tc.tile_pool(name="wk", bufs=2) as wpool, \
                 tc.tile_pool(name="psr", bufs=2, space="PSUM") as psr:
                gt = blocks_of(g_chunked(rpool, idx_r2, NR2, recv2, "r2"))

                def epi(ch, ps):
                    xo = wpool.tile([128, D], f32, tag="xo")
                    nc.sync.dma_start(out=xo[:],
                                      in_=xw[ch * 128:(ch + 1) * 128, :])
                    xn = wpool.tile([128, D], f32, tag="xn")
                    nc.vector.scalar_tensor_tensor(
                        out=xn[:], in0=ps[:],
                        scalar=inv_t[:][:, ch:ch + 1],
                        in1=xo[:], op0=ALU.mult, op1=ALU.add)
                    nc.vector.tensor_tensor(out=xn[:], in0=xn[:],
                                            in1=bobc_t[:], op=ALU.add)
                    nc.sync.dma_start(out=xw[ch * 128:(ch + 1) * 128, :],
                                      in_=xn[:])
                seg_reduce(wpool, psr, gt, dmod1_t, LO1, NCH, epi, "seg1")

                gtc = blocks_of(g_chunked(rpool, idx_cf, NCF, xw, "cfg"))

                def cfcb(ch, ps):
                    cfs = wpool.tile([128, D], f32, tag="cfs")
                    nc.vector.tensor_copy(out=cfs[:], in_=ps[:])
                    nc.sync.dma_start(
                        out=cfpart[ch * 128:(ch + 1) * 128, :], in_=cfs[:])
                seg_reduce(wpool, psr, gtc, dmodc_t, LOC, NCC, cfcb, "seg2")
        collective("AllReduce", ALU.add, [cfpart.ap().opt()],
                   [cfall.ap().opt()])

        # ---------- TC-D: gat2 edges ----------
        with PhaseTC(nc) as tc:
            with tc.tile_pool(name="g2", bufs=2) as pool, \
                 tc.tile_pool(name="psc", bufs=2, space="PSUM") as psc:
                cft = pool.tile([64, 1024], f32, tag="cft", bufs=1)
                for j in range(8):
                    cfin = pool.tile([128, D], f32, tag="cfin")
                    nc.sync.dma_start(out=cfin[:],
                                      in_=cfall[j * 128:(j + 1) * 128, :])
                    tp = psc.tile([64, 128], f32, tag="cftp", space="PSUM")
                    nc.tensor.transpose(out=tp[:], in_=cfin[:],
                                        identity=ident_t[:])
                    nc.vector.tensor_scalar(
                        out=cft[:, j * 128:(j + 1) * 128], in0=tp[:],
                        scalar1=1.0 / VC, scalar2=None, op0=ALU.mult)
                for (wt, dst) in ((wq2_t, q2b), (wk2_t, k2b), (wv2_t, v2b)):
                    for j in range(8):
                        pp = psc.tile([128, D], f32, tag="qkv", space="PSUM")
                        nc.tensor.matmul(out=pp[:],
                                         lhsT=cft[:, j * 128:(j + 1) * 128],
                                         rhs=wt[:], start=True, stop=True)
                        ss = pool.tile([128, D], f32, tag="qkvs")
                        nc.vector.tensor_copy(out=ss[:], in_=pp[:])
                        nc.sync.dma_start(out=dst[j * 128:(j + 1) * 128, :],
                                          in_=ss[:])
                qe = gather(pool, idx_qe, 0, 1024, q2b, "qet", 0)
                ke = gather(pool, idx_ke, 0, 1024, k2b, "ket", 1)
                ve = gather(pool, idx_ke, 0, 1024, v2b, "vet", 2)
                a_t = pool.tile([128, 8], f32, tag="a")
                for j in range(8):
                    tmp = pool.tile([128, D], f32, tag="tmp")
                    nc.vector.tensor_tensor(
                        out=tmp[:], in0=qe[:][:, j * D:(j + 1) * D],
                        in1=ke[:][:, j * D:(j + 1) * D], op=ALU.mult)
                    nc.vector.tensor_reduce(
                        out=a_t[:, j:j + 1], in_=tmp[:],
                        axis=mybir.AxisListType.X, op=ALU.add)
                lr = pool.tile([128, 8], f32, tag="alr")
                nc.vector.tensor_scalar(out=lr[:], in0=a_t[:], scalar1=NEG,
                                        scalar2=None, op0=ALU.mult)
                nc.vector.tensor_tensor(out=lr[:], in0=lr[:], in1=a_t[:],
                                        op=ALU.max)
                sg = pool.tile([128, 8], f32, tag="asg")
                nc.scalar.activation(out=sg[:], in_=lr[:], func=AF.Sigmoid)
                ust = pool.tile([128, 8 * D], f32, tag="ust")
                for j in range(8):
                    nc.vector.tensor_scalar(
                        out=ust[:, j * D:(j + 1) * D],
                        in0=ve[:][:, j * D:(j + 1) * D],
                        scalar1=sg[:, j:j + 1], scalar2=None, op0=ALU.mult)
                nc.sync.dma_start(
                    out=u_buf[:, :].rearrange("(b p) d -> p b d", p=128),
                    in_=ust[:].rearrange("p (b d) -> p b d", d=D))
                off = 0
                qi = 0
                while off < NCORE * B3:
                    n = min(GCAP, NCORE * B3 - off)
                    t = gather(pool, idx_s3, off, n, u_buf, "s3t", qi % 4)
                    nc.sync.dma_start(
                        out=send3[off:off + n, :].rearrange(
                            "(b p) d -> p b d", p=128),
                        in_=t[:][:, :(n // 128) * D].rearrange(
                            "p (b d) -> p b d", d=D))
                    off += n
                    qi += 1
        collective("AllToAll", ALU.bypass, [send3.ap().opt()],
                   [recv3.ap().opt()])

        # ---------- TC-E: gat2 reduce ----------
        with PhaseTC(nc) as tc:
            with tc.tile_pool(name="r3", bufs=1) as rpool, \
                 tc.tile_pool(name="w3", bufs=2) as wpool, \
                 tc.tile_pool(name="ps3", bufs=2, space="PSUM") as ps3:
                gt3 = blocks_of(g_chunked(rpool, idx_r3, NR3, recv3, "r3"))

                def upd(ch, ps):
                    xo = wpool.tile([128, D], f32, tag="xo3")
                    nc.sync.dma_start(out=xo[:],
                                      in_=xw[ch * 128:(ch + 1) * 128, :])
                    xn = wpool.tile([128, D], f32, tag="xn3")
                    nc.vector.tensor_tensor(out=xn[:], in0=ps[:], in1=xo[:],
                                            op=ALU.add)
                    nc.sync.dma_start(out=xw[ch * 128:(ch + 1) * 128, :],
                                      in_=xn[:])
                seg_reduce(wpool, ps3, gt3, dmod3_t, LO3, NVCH, upd, "seg3")

    # ---------- readout ----------
    with PhaseTC(nc) as tc:
        with tc.tile_pool(name="ro", bufs=3) as pool, \
             tc.tile_pool(name="psm", bufs=1, space="PSUM") as psm:
            rmask_t = pool.tile([128, 2 * NCH], f32, tag="rm", bufs=1)
            nc.sync.dma_start(out=rmask_t[:], in_=rmask_d[:, :])
            sp = psm.tile([2, D], f32, tag="sums", space="PSUM")
            for ch in range(NCH):
                xt = pool.tile([128, D], f32, tag="xt")
                nc.sync.dma_start(out=xt[:],
                                  in_=xw[ch * 128:(ch + 1) * 128, :])
                nc.tensor.matmul(out=sp[:],
                                 lhsT=rmask_t[:, 2 * ch:2 * ch + 2],
                                 rhs=xt[:], start=(ch == 0),
                                 stop=(ch == NCH - 1))
            ssb = pool.tile([2, D], f32, tag="ssb")
            nc.vector.tensor_copy(out=ssb[:], in_=sp[:])
            nc.sync.dma_start(out=sums_d[:, :], in_=ssb[:])
    collective("AllReduce", ALU.add, [sums_d.ap().opt()], [sums_a.ap().opt()])
    with PhaseTC(nc) as tc:
        with tc.tile_pool(name="fin", bufs=1) as pool, \
             tc.tile_pool(name="psf", bufs=1, space="PSUM") as psf:
            sm = pool.tile([2, D], f32, tag="sm")
            nc.sync.dma_start(out=sm[:], in_=sums_a[:, :])
            sc = pool.tile([2, 1], f32, tag="sc")
            nc.sync.dma_start(out=sc[:], in_=scale2_d[:, :])
            mn = pool.tile([2, D], f32, tag="mn")
            nc.vector.tensor_scalar(out=mn[:], in0=sm[:], scalar1=sc[:, 0:1],
                                    scalar2=None, op0=ALU.mult)
            th = pool.tile([2, D], f32, tag="th")
            nc.scalar.activation(out=th[:], in_=mn[:], func=AF.Tanh)
            tp = psf.tile([64, 2], f32, tag="tp", space="PSUM")
            nc.tensor.transpose(out=tp[:, 0:2], in_=th[:],
                                identity=ident_t[:][0:2, 0:2])
            tsb = pool.tile([64, 2], f32, tag="tsb")
            nc.vector.tensor_copy(out=tsb[:], in_=tp[:, 0:2])
            pooled = pool.tile([128, 1], f32, tag="pooled")
            nc.sync.dma_start(out=pooled[0:64, :], in_=tsb[:, 0:1])
            nc.sync.dma_start(out=pooled[64:128, :], in_=tsb[:, 1:2])
            w1t = pool.tile([128, HID], f32, tag="w1t")
            nc.sync.dma_start(out=w1t[:], in_=W1T[:, :])
            b1s = pool.tile([HID, 1], f32, tag="b1s")
            nc.sync.dma_start(out=b1s[:], in_=b1t[:, :])
            hp = psf.tile([HID, 1], f32, tag="hp", space="PSUM")
            nc.tensor.matmul(out=hp[:], lhsT=w1t[:], rhs=pooled[:],
                             start=True, stop=True)
            hs = pool.tile([HID, 1], f32, tag="hs")
            nc.scalar.activation(out=hs[:], in_=hp[:], func=AF.Relu,
                                 bias=b1s[:])
            w2t = pool.tile([HID, 1], f32, tag="w2t")
            nc.sync.dma_start(out=w2t[:], in_=W2T[:, :])
            op = psf.tile([1, 1], f32, tag="op", space="PSUM")
            nc.tensor.matmul(out=op[:], lhsT=w2t[:], rhs=hs[:], start=True,
                             stop=True)
            b2s = pool.tile([1, 1], f32, tag="b2s")
            nc.sync.dma_start(out=b2s[:], in_=b2t[:, :])
            ores = pool.tile([1, 1], f32, tag="ores")
            nc.vector.tensor_tensor(out=ores[:], in0=op[:], in1=b2s[:],
                                    op=ALU.add)
            nc.sync.dma_start(out=y[:, :], in_=ores[:])

    nc.compile()
    return nc


NAMES = ["xsl0", "serve1", "hop2", "send2g", "r2g", "dmod1", "invcnt",
         "cfg", "dmodc", "qeg", "keg", "send3g", "r3g", "dmod3", "biasA",
         "biasB", "biasA2", "biasB2", "bo_bc", "WQE", "WKE", "WVE", "WoTe", "Wq2T", "Wk2T",
         "Wv2T", "W1T", "b1", "W2T", "b2", "ident", "iota", "scale2",
         "rmask"]


def kernel(**inputs):
    from concourse.bass_utils import run_bass_kernel_spmd
    dims, per = _plan(inputs)
    nc = _build(dims)
    in_maps = [{n: np.ascontiguousarray(per[k][n]) for n in NAMES}
               for k in range(NCORE)]
    res = run_bass_kernel_spmd(nc, in_maps, list(range(NCORE)))
    return np.asarray(res.results[0]["y"]).reshape(1).astype(np.float32)

